# revision 13
# baseline (speedup 1.0000x reference)
"""Single-head attention (B=4, S=2048, E=1024, H=64, fp32) on 8 TRN2 NeuronCores.

Sharding: each batch b is handled by a core pair; core 2b takes keys/values
[0:1024), core 2b+1 takes [1024:2048) (ring-attention-style split over the
key axis, per the sharding hint). Each core computes, for ALL 2048 queries of
its batch, the unnormalized attention numerator and denominator over its key
half; the host sums the two halves and divides (the cross-shard combine).

Per core:
  - kT-half/qT/vT-half projections [64, *] via lhsT=W-chunk (64-col
    stationary), rhs = x.T chunks (512-wide moving) accumulated over E in
    PSUM; the three x.T inputs are concatenated host-side into one
    [8, 128, 4096] tensor, DMA'd as 1 MB column-halves so compute starts
    early.
  - v is re-materialized [sk, 64+1] via PE transposes of vT (ones column
    appended for the denominator).
  - scores transposed [sk, sq] = (kT-slice).T @ qT -> PSUM -> ACT exp ->
    SBUF. Unnormalized softmax: scores are bounded ~ +-50, exp <= ~1e21,
    safe in fp32. Optionally two K=64 score matmuls are packed into the
    128-row PE array concurrently via tile_position row groups.
  - AV: outT [65, 512] += ([v|1]).T @ expT-chunk accumulated over sk-chunks
    in PSUM; row 64 is the denominator. Raw [65, 2048] goes back to the
    host.

All heavy matmuls keep the stationary operand small (<=65 cols) and the
moving operand 512 wide, so the fp32 two-pass weight loads hide under the
streaming and the PE stays HAM-warm.
"""

import numpy as np

_B, _S, _E, _H = 4, 2048, 1024, 64
_P = 128
_EC = _E // _P          # 8 E-chunks
_SK = _S // 2           # 1024 keys per core
_SKC = _SK // _P        # 8 sk chunks
_W = _SK + _S + _SK     # 4096 combined x.T width (kT | qT | vT)
_NJ = _W // 512         # 8 projection col-chunks (2 kT, 4 qT, 2 vT)
_HW = _W // 2           # 2048 col-half width
_NH = _S // 512         # 4 query 512-chunks

# per-stage matmul dtypes ("float32" or "float32r")
_DT_KQ = "float32"      # kT/qT/vT projection matmuls
_DT_SC = "float32"      # scores matmuls
_DT_AV = "float32"      # attention-weighted value matmuls
_DT_TR = "float32"      # PE transposes (v)
_SC_PACK = True         # pack pairs of K=64 score matmuls into row groups
_PJ_PACK = True         # pack pairs of M=64 projection matmuls into col groups

_built = None


def _mmdt(name):
    import concourse.mybir as mybir
    return getattr(mybir.dt, name)


def _build():
    import concourse.bacc as bacc
    import concourse.mybir as mybir
    import concourse.tile as tile

    f32 = mybir.dt.float32
    Exp = mybir.ActivationFunctionType.Exp
    Ident = mybir.ActivationFunctionType.Identity

    nc = bacc.Bacc("TRN2", target_bir_lowering=False, debug=False,
                   enable_asserts=False, num_devices=8)
    dt_kq = _mmdt(_DT_KQ)
    dt_sc = _mmdt(_DT_SC)
    dt_av = _mmdt(_DT_AV)
    dt_tr = _mmdt(_DT_TR)

    x_d = nc.dram_tensor("x", [_EC, _P, _W], dt_kq, kind="ExternalInput")
    w_d = nc.dram_tensor("w", [_P, 3, _EC, _H], dt_kq, kind="ExternalInput")
    bs_d = nc.dram_tensor("bs", [_H, 3], f32, kind="ExternalInput")
    id_d = nc.dram_tensor("ident", [_H, _H], dt_tr, kind="ExternalInput")
    out_d = nc.dram_tensor("out", [_H + 1, _S], f32, kind="ExternalOutput")

    # scores operand rows: with row-packing, kqT is duplicated on
    # partitions 64..127 so a second matmul can run in the lower PE rows
    nrow = 2 if _SC_PACK else 1

    with tile.TileContext(nc) as tc:
        with (
            tc.tile_pool(name="persist", bufs=1) as persist,
            tc.tile_pool(name="xa_p", bufs=4) as xa_p,
            tc.tile_pool(name="xb_p", bufs=4) as xb_p,
        ):
            w_sb = persist.tile([_P, 3, _EC, _H], dt_kq)
            bs_sb = persist.tile([_H, 3], f32)
            id_sb = persist.tile([_H, _H], dt_tr)
            # wk first: it is all the first projection matmuls need
            nc.scalar.dma_start(w_sb[:, 0], w_d.ap()[:, 0])
            nc.scalar.dma_start(w_sb[:, 1], w_d.ap()[:, 1])
            nc.scalar.dma_start(w_sb[:, 2], w_d.ap()[:, 2])
            nc.scalar.dma_start(bs_sb[:], bs_d.ap())
            nc.scalar.dma_start(id_sb[:], id_d.ap())

            kqT_sb = persist.tile([nrow * _H, _SK + _S], dt_sc)  # kT | qT
            vT_sb = persist.tile([_H, _SK], dt_tr)
            v_sb = persist.tile([_P, _SKC, _H + 1], dt_av)  # v with ones col
            oT_sb = persist.tile([_H + 1, _S], f32)

            nc.vector.memset(v_sb[:, :, _H:_H + 1], 1.0)

            # ---- phase 1: projections (kT/qT/vT) + v rebuild ----
            # j -> (weight idx, psum tile, partition half)
            jmap = [(0, 0, 0), (0, 1, 0),                         # kT
                    (1, 2, 0), (1, 2, 1), (1, 3, 0), (1, 3, 1),   # qT
                    (2, 0, 1), (2, 1, 1)]                          # vT
            # copy order: unblock scores (kT j0, qT j2) and v transposes first
            jcopy = [0, 2, 6, 7, 3, 4, 5, 1]

            def proj_copy(j):
                wi, pt, half = jmap[j]
                src = psums[pt][half * _H:(half + 1) * _H, :]
                b = bs_sb[:, wi:wi + 1]
                if j < 2:
                    dsts = [kqT_sb[r * _H:(r + 1) * _H, j * 512:(j + 1) * 512]
                            for r in range(nrow)]
                elif j < 6:
                    dsts = [kqT_sb[r * _H:(r + 1) * _H,
                                   _SK + (j - 2) * 512:_SK + (j - 1) * 512]
                            for r in range(nrow)]
                else:
                    dsts = [vT_sb[:, (j - 6) * 512:(j - 5) * 512]]
                # split the copy-back between DVE and ACT so the
                # projection->scores handoff isn't serialized on one engine
                for r, dst in enumerate(dsts):
                    if (j + r) % 2 == 0:
                        nc.vector.tensor_scalar_add(dst, src, b)
                    else:
                        nc.scalar.activation(dst, src, Ident, bias=b)

            with (
                tc.tile_pool(name="mm_ps", bufs=4, space="PSUM") as mm_ps,
                tc.tile_pool(name="vt_ps", bufs=2, space="PSUM") as vt_ps,
                tc.tile_pool(name="av_ps", bufs=2, space="PSUM") as av_ps,
                tc.tile_pool(name="expT_p", bufs=1) as expT_p,
            ):
                psums = [mm_ps.tile([_P, 512], f32, name=f"pkq{t}", tag="ps")
                         for t in range(4)]
                xa_tiles, xb_tiles = [], []
                for e in range(_EC):
                    ta = xa_p.tile([_P, _HW], dt_kq, name="xa_t", tag="xa_t")
                    if e == 0:
                        nc.sync.dma_start(ta[:, :512], x_d.ap()[e, :, :512])
                        nc.sync.dma_start(ta[:, 512:], x_d.ap()[e, :, 512:_HW])
                    else:
                        nc.sync.dma_start(ta[:], x_d.ap()[e, :, : _HW])
                    xa_tiles.append(ta)
                    tb = xb_p.tile([_P, _HW], dt_kq, name="xb_t", tag="xb_t")
                    nc.sync.dma_start(tb[:], x_d.ap()[e, :, _HW:])
                    xb_tiles.append(tb)

                def rhs(e, j):
                    if j < 4:
                        return xa_tiles[e][:, j * 512:(j + 1) * 512]
                    return xb_tiles[e][:, (j - 4) * 512:(j - 3) * 512]

                for e in range(_EC):
                    order = jcopy if e == _EC - 1 else range(_NJ)
                    for j in order:
                        wi, pt, half = jmap[j]
                        nc.tensor.matmul(
                            psums[pt][half * _H:(half + 1) * _H, :],
                            w_sb[:, wi, e, :],
                            rhs(e, j),
                            start=(e == 0), stop=(e == _EC - 1),
                            tile_position=(0, half * _H) if _PJ_PACK else None,
                            # top/bottom halves of one bank hold independent
                            # accumulation groups (disjoint partitions); the
                            # sim's zero-region tracker can't see that
                            skip_group_check=True,
                        )
                        if e == _EC - 1:
                            proj_copy(j)

                # v[sk, h] via PE transpose of vT
                for skc in range(_SKC):
                    pvt = vt_ps.tile([_P, _H], f32, name="pvt")
                    nc.tensor.transpose(
                        pvt[:],
                        vT_sb[:, skc * _P:(skc + 1) * _P],
                        id_sb[:],
                    )
                    nc.vector.tensor_copy(v_sb[:, skc, : _H], pvt[:])

                # ---- phase 2: scores + exp + AV ----
                expT = expT_p.tile([_P, _SKC, _S], dt_av)
                for skc in range(0, _SKC, nrow):
                    for h in range(_NH):
                        for r in range(nrow):
                            sk = skc + r
                            ps = mm_ps.tile([_P, 512], f32, name="ps", tag="ps")
                            nc.tensor.matmul(
                                ps[:],
                                kqT_sb[r * _H:(r + 1) * _H,
                                       sk * _P:(sk + 1) * _P],
                                kqT_sb[r * _H:(r + 1) * _H,
                                       _SK + h * 512:_SK + (h + 1) * 512],
                                start=True, stop=True,
                                tile_position=(r * _H, 0) if _SC_PACK else None,
                            )
                            nc.scalar.activation(
                                expT[:, sk, h * 512:(h + 1) * 512], ps[:], Exp
                            )

                for h in range(_NH):
                    po = av_ps.tile([_H + 1, 512], f32, name="po")
                    for skc in range(_SKC):
                        nc.tensor.matmul(
                            po[:],
                            v_sb[:, skc, :],
                            expT[:, skc, h * 512:(h + 1) * 512],
                            start=(skc == 0), stop=(skc == _SKC - 1),
                        )
                    nc.vector.tensor_copy(oT_sb[:, h * 512:(h + 1) * 512], po[:])
                    nc.sync.dma_start(
                        out_d.ap()[:, h * 512:(h + 1) * 512],
                        oT_sb[:, h * 512:(h + 1) * 512],
                    )

    nc.compile()
    return nc


def _prep_core(query, key, value, Wq, bq, Wk, bk, Wv, bv, core):
    b, half = core // 2, core % 2
    xkT = np.ascontiguousarray(key[b].T[:, half * _SK:(half + 1) * _SK])
    xqT = np.ascontiguousarray(query[b].T)                 # [E, S]
    xvT = np.ascontiguousarray(value[b].T[:, half * _SK:(half + 1) * _SK])
    x = np.concatenate(
        [xkT.reshape(_EC, _P, _SK), xqT.reshape(_EC, _P, _S),
         xvT.reshape(_EC, _P, _SK)], axis=2,
    )                                                      # [EC, P, W]
    w = np.stack(
        [Wk.reshape(_EC, _P, _H), Wq.reshape(_EC, _P, _H),
         Wv.reshape(_EC, _P, _H)], axis=0,
    ).transpose(2, 0, 1, 3)                                # [P, 3, EC, H]
    bs = np.stack(
        [np.asarray(bk, dtype=np.float32).ravel(),
         np.asarray(bq, dtype=np.float32).ravel(),
         np.asarray(bv, dtype=np.float32).ravel()], axis=1,
    )                                                      # [H, 3]
    return {
        "x": np.ascontiguousarray(x, dtype=np.float32),
        "w": np.ascontiguousarray(w, dtype=np.float32),
        "bs": np.ascontiguousarray(bs, dtype=np.float32),
        "ident": np.eye(_H, dtype=np.float32),
    }


def _get_built():
    global _built
    if _built is None:
        _built = _build()
    return _built


def kernel(query, key, value, Wq, bq, Wk, bk, Wv, bv, _trace=False):
    from concourse.bass_utils import run_bass_kernel_spmd

    query = np.asarray(query, dtype=np.float32)
    key = np.asarray(key, dtype=np.float32)
    value = np.asarray(value, dtype=np.float32)
    Wq = np.asarray(Wq, dtype=np.float32)
    Wk = np.asarray(Wk, dtype=np.float32)
    Wv = np.asarray(Wv, dtype=np.float32)

    nc = _get_built()
    in_maps = [
        _prep_core(query, key, value, Wq, bq, Wk, bk, Wv, bv, c) for c in range(8)
    ]
    res = run_bass_kernel_spmd(nc, in_maps, core_ids=list(range(8)), trace=_trace)
    out = np.empty((_B, _S, _H), dtype=np.float32)
    for b in range(_B):
        oA = res.results[2 * b]["out"]      # [H+1, S]
        oB = res.results[2 * b + 1]["out"]
        num = oA[: _H] + oB[: _H]
        den = oA[_H] + oB[_H]
        out[b] = (num / den).T
    if _trace:
        kernel.last_result = res
    return out


# revision 14
# speedup vs baseline: 1.0470x; 1.0470x over previous
"""Single-head attention (B=4, S=2048, E=1024, H=64, fp32) on 8 TRN2 NeuronCores.

Sharding: each batch b is handled by a core pair; core 2b takes keys/values
[0:1024), core 2b+1 takes [1024:2048) (ring-attention-style split over the
key axis, per the sharding hint). Each core computes, for ALL 2048 queries of
its batch, the unnormalized attention numerator and denominator over its key
half; the host sums the two halves and divides (the cross-shard combine).

Per core:
  - kT-half/qT/vT-half projections [64, *] via lhsT=W-chunk (64-col
    stationary), rhs = x.T chunks (512-wide moving) accumulated over E in
    PSUM; the three x.T inputs are concatenated host-side into one
    [8, 128, 4096] tensor, DMA'd as 1 MB column-halves so compute starts
    early.
  - v is re-materialized [sk, 64+1] via PE transposes of vT (ones column
    appended for the denominator).
  - scores transposed [sk, sq] = (kT-slice).T @ qT -> PSUM -> ACT exp ->
    SBUF. Unnormalized softmax: scores are bounded ~ +-50, exp <= ~1e21,
    safe in fp32. Optionally two K=64 score matmuls are packed into the
    128-row PE array concurrently via tile_position row groups.
  - AV: outT [65, 512] += ([v|1]).T @ expT-chunk accumulated over sk-chunks
    in PSUM; row 64 is the denominator. Raw [65, 2048] goes back to the
    host.

All heavy matmuls keep the stationary operand small (<=65 cols) and the
moving operand 512 wide, so the fp32 two-pass weight loads hide under the
streaming and the PE stays HAM-warm.
"""

import numpy as np

_B, _S, _E, _H = 4, 2048, 1024, 64
_P = 128
_EC = _E // _P          # 8 E-chunks
_SK = _S // 2           # 1024 keys per core
_SKC = _SK // _P        # 8 sk chunks
_W = _SK + _S + _SK     # 4096 combined x.T width (kT | qT | vT)
_NJ = _W // 512         # 8 projection col-chunks (2 kT, 4 qT, 2 vT)
_HW = _W // 2           # 2048 col-half width
_NH = _S // 512         # 4 query 512-chunks

# per-stage matmul dtypes ("float32" or "float32r")
_DT_KQ = "float32"      # kT/qT/vT projection matmuls
_DT_SC = "float32"      # scores matmuls
_DT_AV = "float32"      # attention-weighted value matmuls
_DT_TR = "float32"      # PE transposes (v)
_SC_PACK = True         # pack pairs of K=64 score matmuls into row groups
_PJ_PACK = True         # pack pairs of M=64 projection matmuls into col groups

_built = None


def _mmdt(name):
    import concourse.mybir as mybir
    return getattr(mybir.dt, name)


def _build():
    import concourse.bacc as bacc
    import concourse.mybir as mybir
    import concourse.tile as tile

    f32 = mybir.dt.float32
    Exp = mybir.ActivationFunctionType.Exp
    Ident = mybir.ActivationFunctionType.Identity

    nc = bacc.Bacc("TRN2", target_bir_lowering=False, debug=False,
                   enable_asserts=False, num_devices=8)
    dt_kq = _mmdt(_DT_KQ)
    dt_sc = _mmdt(_DT_SC)
    dt_av = _mmdt(_DT_AV)
    dt_tr = _mmdt(_DT_TR)

    x_d = nc.dram_tensor("x", [_EC, _P, _W], dt_kq, kind="ExternalInput")
    w_d = nc.dram_tensor("w", [_P, 3, _EC, _H], dt_kq, kind="ExternalInput")
    bs_d = nc.dram_tensor("bs", [_H, 3], f32, kind="ExternalInput")
    id_d = nc.dram_tensor("ident", [_H, _H], dt_tr, kind="ExternalInput")
    out_d = nc.dram_tensor("out", [_H + 1, _S], f32, kind="ExternalOutput")

    # scores operand rows: with row-packing, kqT is duplicated on
    # partitions 64..127 so a second matmul can run in the lower PE rows
    nrow = 2 if _SC_PACK else 1

    with tile.TileContext(nc) as tc:
        with (
            tc.tile_pool(name="persist", bufs=1) as persist,
            tc.tile_pool(name="xa_p", bufs=4) as xa_p,
            tc.tile_pool(name="xb_p", bufs=4) as xb_p,
        ):
            w_sb = persist.tile([_P, 3, _EC, _H], dt_kq)
            bs_sb = persist.tile([_H, 3], f32)
            id_sb = persist.tile([_H, _H], dt_tr)
            # wk first: it is all the first projection matmuls need
            nc.scalar.dma_start(w_sb[:, 0], w_d.ap()[:, 0])
            nc.scalar.dma_start(w_sb[:, 1], w_d.ap()[:, 1])
            nc.scalar.dma_start(w_sb[:, 2], w_d.ap()[:, 2])
            nc.scalar.dma_start(bs_sb[:], bs_d.ap())
            nc.scalar.dma_start(id_sb[:], id_d.ap())

            kqT_sb = persist.tile([nrow * _H, _SK + _S], dt_sc)  # kT | qT
            vT_sb = persist.tile([_H, _SK], dt_tr)
            v_sb = persist.tile([_P, _SKC, _H + 1], dt_av)  # v with ones col
            oT_sb = persist.tile([_H + 1, _S], f32)

            nc.vector.memset(v_sb[:, :, _H:_H + 1], 1.0)

            # ---- phase 1: projections (kT/qT/vT) + v rebuild ----
            # j -> (weight idx, psum tile, partition half)
            jmap = [(0, 0, 0), (0, 1, 0),                         # kT
                    (1, 2, 0), (1, 2, 1), (1, 3, 0), (1, 3, 1),   # qT
                    (2, 0, 1), (2, 1, 1)]                          # vT
            # copy order: unblock scores (kT j0, qT j2) and v transposes first
            jcopy = [0, 6, 2, 3, 4, 5, 7, 1]
            jpair = [0, 6, 1, 7, 2, 3, 4, 5]

            def proj_copy(j):
                wi, pt, half = jmap[j]
                src = psums[pt][half * _H:(half + 1) * _H, :]
                b = bs_sb[:, wi:wi + 1]
                if j < 2:
                    dsts = [kqT_sb[r * _H:(r + 1) * _H, j * 512:(j + 1) * 512]
                            for r in range(nrow)]
                elif j < 6:
                    dsts = [kqT_sb[r * _H:(r + 1) * _H,
                                   _SK + (j - 2) * 512:_SK + (j - 1) * 512]
                            for r in range(nrow)]
                else:
                    dsts = [vT_sb[:, (j - 6) * 512:(j - 5) * 512]]
                # split the copy-back between DVE and ACT so the
                # projection->scores handoff isn't serialized on one engine
                for r, dst in enumerate(dsts):
                    if (j + r) % 2 == 0:
                        nc.vector.tensor_scalar_add(dst, src, b)
                    else:
                        nc.scalar.activation(dst, src, Ident, bias=b)

            with (
                tc.tile_pool(name="mm_ps", bufs=4, space="PSUM") as mm_ps,
                tc.tile_pool(name="vt_ps", bufs=2, space="PSUM") as vt_ps,
                tc.tile_pool(name="av_ps", bufs=2, space="PSUM") as av_ps,
                tc.tile_pool(name="expT_p", bufs=1) as expT_p,
            ):
                psums = [mm_ps.tile([_P, 512], f32, name=f"pkq{t}", tag="ps")
                         for t in range(4)]
                xa_tiles, xb_tiles = [], []
                for e in range(_EC):
                    ta = xa_p.tile([_P, _HW], dt_kq, name="xa_t", tag="xa_t")
                    if e == 0:
                        nc.sync.dma_start(ta[:, :512], x_d.ap()[e, :, :512])
                        nc.sync.dma_start(ta[:, 512:], x_d.ap()[e, :, 512:_HW])
                    else:
                        nc.sync.dma_start(ta[:], x_d.ap()[e, :, : _HW])
                    xa_tiles.append(ta)
                    tb = xb_p.tile([_P, _HW], dt_kq, name="xb_t", tag="xb_t")
                    nc.sync.dma_start(tb[:], x_d.ap()[e, :, _HW:])
                    xb_tiles.append(tb)

                def rhs(e, j):
                    if j < 4:
                        return xa_tiles[e][:, j * 512:(j + 1) * 512]
                    return xb_tiles[e][:, (j - 4) * 512:(j - 3) * 512]

                for e in range(_EC):
                    order = jcopy if e == _EC - 1 else jpair
                    for j in order:
                        wi, pt, half = jmap[j]
                        nc.tensor.matmul(
                            psums[pt][half * _H:(half + 1) * _H, :],
                            w_sb[:, wi, e, :],
                            rhs(e, j),
                            start=(e == 0), stop=(e == _EC - 1),
                            tile_position=(0, half * _H) if _PJ_PACK else None,
                            # top/bottom halves of one bank hold independent
                            # accumulation groups (disjoint partitions); the
                            # sim's zero-region tracker can't see that
                            skip_group_check=True,
                        )
                        if e == _EC - 1:
                            proj_copy(j)

                # v[sk, h] via PE transpose of vT
                for skc in range(_SKC):
                    pvt = vt_ps.tile([_P, _H], f32, name="pvt")
                    nc.tensor.transpose(
                        pvt[:],
                        vT_sb[:, skc * _P:(skc + 1) * _P],
                        id_sb[:],
                    )
                    nc.vector.tensor_copy(v_sb[:, skc, : _H], pvt[:])

                # ---- phase 2: scores + exp + AV ----
                expT = expT_p.tile([_P, _SKC, _S], dt_av)
                for skc in range(0, _SKC, nrow):
                    for h in range(_NH):
                        for r in range(nrow):
                            sk = skc + r
                            ps = mm_ps.tile([_P, 512], f32, name="ps", tag="ps")
                            nc.tensor.matmul(
                                ps[:],
                                kqT_sb[r * _H:(r + 1) * _H,
                                       sk * _P:(sk + 1) * _P],
                                kqT_sb[r * _H:(r + 1) * _H,
                                       _SK + h * 512:_SK + (h + 1) * 512],
                                start=True, stop=True,
                                tile_position=(r * _H, 0) if _SC_PACK else None,
                            )
                            nc.scalar.activation(
                                expT[:, sk, h * 512:(h + 1) * 512], ps[:], Exp
                            )

                for h in range(_NH):
                    po = av_ps.tile([_H + 1, 512], f32, name="po")
                    for skc in range(_SKC):
                        nc.tensor.matmul(
                            po[:],
                            v_sb[:, skc, :],
                            expT[:, skc, h * 512:(h + 1) * 512],
                            start=(skc == 0), stop=(skc == _SKC - 1),
                        )
                    nc.vector.tensor_copy(oT_sb[:, h * 512:(h + 1) * 512], po[:])
                    nc.sync.dma_start(
                        out_d.ap()[:, h * 512:(h + 1) * 512],
                        oT_sb[:, h * 512:(h + 1) * 512],
                    )

    nc.compile()
    return nc


def _prep_core(query, key, value, Wq, bq, Wk, bk, Wv, bv, core):
    b, half = core // 2, core % 2
    xkT = np.ascontiguousarray(key[b].T[:, half * _SK:(half + 1) * _SK])
    xqT = np.ascontiguousarray(query[b].T)                 # [E, S]
    xvT = np.ascontiguousarray(value[b].T[:, half * _SK:(half + 1) * _SK])
    x = np.concatenate(
        [xkT.reshape(_EC, _P, _SK), xqT.reshape(_EC, _P, _S),
         xvT.reshape(_EC, _P, _SK)], axis=2,
    )                                                      # [EC, P, W]
    w = np.stack(
        [Wk.reshape(_EC, _P, _H), Wq.reshape(_EC, _P, _H),
         Wv.reshape(_EC, _P, _H)], axis=0,
    ).transpose(2, 0, 1, 3)                                # [P, 3, EC, H]
    bs = np.stack(
        [np.asarray(bk, dtype=np.float32).ravel(),
         np.asarray(bq, dtype=np.float32).ravel(),
         np.asarray(bv, dtype=np.float32).ravel()], axis=1,
    )                                                      # [H, 3]
    return {
        "x": np.ascontiguousarray(x, dtype=np.float32),
        "w": np.ascontiguousarray(w, dtype=np.float32),
        "bs": np.ascontiguousarray(bs, dtype=np.float32),
        "ident": np.eye(_H, dtype=np.float32),
    }


def _get_built():
    global _built
    if _built is None:
        _built = _build()
    return _built


def kernel(query, key, value, Wq, bq, Wk, bk, Wv, bv, _trace=False):
    from concourse.bass_utils import run_bass_kernel_spmd

    query = np.asarray(query, dtype=np.float32)
    key = np.asarray(key, dtype=np.float32)
    value = np.asarray(value, dtype=np.float32)
    Wq = np.asarray(Wq, dtype=np.float32)
    Wk = np.asarray(Wk, dtype=np.float32)
    Wv = np.asarray(Wv, dtype=np.float32)

    nc = _get_built()
    in_maps = [
        _prep_core(query, key, value, Wq, bq, Wk, bk, Wv, bv, c) for c in range(8)
    ]
    res = run_bass_kernel_spmd(nc, in_maps, core_ids=list(range(8)), trace=_trace)
    out = np.empty((_B, _S, _H), dtype=np.float32)
    for b in range(_B):
        oA = res.results[2 * b]["out"]      # [H+1, S]
        oB = res.results[2 * b + 1]["out"]
        num = oA[: _H] + oB[: _H]
        den = oA[_H] + oB[_H]
        out[b] = (num / den).T
    if _trace:
        kernel.last_result = res
    return out


# revision 15
# speedup vs baseline: 1.0659x; 1.0181x over previous
"""Single-head attention (B=4, S=2048, E=1024, H=64, fp32) on 8 TRN2 NeuronCores.

Sharding: each batch b is handled by a core pair; core 2b takes keys/values
[0:1024), core 2b+1 takes [1024:2048) (ring-attention-style split over the
key axis, per the sharding hint). Each core computes, for ALL 2048 queries of
its batch, the unnormalized attention numerator and denominator over its key
half; the host sums the two halves and divides (the cross-shard combine).

Per core:
  - kT-half/qT/vT-half projections [64, *] via lhsT=W-chunk (64-col
    stationary), rhs = x.T chunks (512-wide moving) accumulated over E in
    PSUM; the three x.T inputs are concatenated host-side into one
    [8, 128, 4096] tensor, DMA'd as 1 MB column-halves so compute starts
    early.
  - v is re-materialized [sk, 64+1] via PE transposes of vT (ones column
    appended for the denominator).
  - scores transposed [sk, sq] = (kT-slice).T @ qT -> PSUM -> ACT exp ->
    SBUF. Unnormalized softmax: scores are bounded ~ +-50, exp <= ~1e21,
    safe in fp32. Optionally two K=64 score matmuls are packed into the
    128-row PE array concurrently via tile_position row groups.
  - AV: outT [65, 512] += ([v|1]).T @ expT-chunk accumulated over sk-chunks
    in PSUM; row 64 is the denominator. Raw [65, 2048] goes back to the
    host.

All heavy matmuls keep the stationary operand small (<=65 cols) and the
moving operand 512 wide, so the fp32 two-pass weight loads hide under the
streaming and the PE stays HAM-warm.
"""

import numpy as np

_B, _S, _E, _H = 4, 2048, 1024, 64
_P = 128
_EC = _E // _P          # 8 E-chunks
_SK = _S // 2           # 1024 keys per core
_SKC = _SK // _P        # 8 sk chunks
_W = _SK + _S + _SK     # 4096 combined x.T width (kT | qT | vT)
_NJ = _W // 512         # 8 projection col-chunks (2 kT, 4 qT, 2 vT)
_HW = _W // 2           # 2048 col-half width
_NH = _S // 512         # 4 query 512-chunks

# per-stage matmul dtypes ("float32" or "float32r")
_DT_KQ = "float32"      # kT/qT/vT projection matmuls
_DT_SC = "float32"      # scores matmuls
_DT_AV = "float32"      # attention-weighted value matmuls
_DT_TR = "float32"      # PE transposes (v)
_SC_PACK = True         # pack pairs of K=64 score matmuls into row groups
_PJ_PACK = False        # col-packing cannot help fp32 (SBUF stream-BW bound)

_built = None


def _mmdt(name):
    import concourse.mybir as mybir
    return getattr(mybir.dt, name)


def _build():
    import concourse.bacc as bacc
    import concourse.mybir as mybir
    import concourse.tile as tile

    f32 = mybir.dt.float32
    Exp = mybir.ActivationFunctionType.Exp
    Ident = mybir.ActivationFunctionType.Identity

    nc = bacc.Bacc("TRN2", target_bir_lowering=False, debug=False,
                   enable_asserts=False, num_devices=8)
    dt_kq = _mmdt(_DT_KQ)
    dt_sc = _mmdt(_DT_SC)
    dt_av = _mmdt(_DT_AV)
    dt_tr = _mmdt(_DT_TR)

    x_d = nc.dram_tensor("x", [_EC, _P, _W], dt_kq, kind="ExternalInput")
    w_d = nc.dram_tensor("w", [_P, 3, _EC, _H], dt_kq, kind="ExternalInput")
    bs_d = nc.dram_tensor("bs", [_H, 3], f32, kind="ExternalInput")
    id_d = nc.dram_tensor("ident", [_H, _H], dt_tr, kind="ExternalInput")
    out_d = nc.dram_tensor("out", [_H + 1, _S], f32, kind="ExternalOutput")

    # scores operand rows: with row-packing, kqT is duplicated on
    # partitions 64..127 so a second matmul can run in the lower PE rows
    nrow = 2 if _SC_PACK else 1

    with tile.TileContext(nc) as tc:
        with (
            tc.tile_pool(name="persist", bufs=1) as persist,
            tc.tile_pool(name="xa_p", bufs=4) as xa_p,
            tc.tile_pool(name="xb_p", bufs=4) as xb_p,
        ):
            w_sb = persist.tile([_P, 3, _EC, _H], dt_kq)
            bs_sb = persist.tile([_H, 3], f32)
            id_sb = persist.tile([_H, _H], dt_tr)
            # wk first: it is all the first projection matmuls need
            nc.scalar.dma_start(w_sb[:, 0], w_d.ap()[:, 0])
            nc.scalar.dma_start(w_sb[:, 1], w_d.ap()[:, 1])
            nc.scalar.dma_start(w_sb[:, 2], w_d.ap()[:, 2])
            nc.scalar.dma_start(bs_sb[:], bs_d.ap())
            nc.scalar.dma_start(id_sb[:], id_d.ap())

            kqT_sb = persist.tile([nrow * _H, _SK + _S], dt_sc)  # kT | qT
            vT_sb = persist.tile([_H, _SK], dt_tr)
            v_sb = persist.tile([_P, _SKC, _H + 1], dt_av)  # v with ones col
            oT_sb = persist.tile([_H + 1, _S], f32)

            nc.vector.memset(v_sb[:, :, _H:_H + 1], 1.0)

            # ---- phase 1: projections (kT/qT/vT) + v rebuild ----
            # j -> (weight idx, psum tile, partition half)
            jmap = [(0, 0, 0), (0, 1, 0),                         # kT
                    (1, 2, 0), (1, 2, 1), (1, 3, 0), (1, 3, 1),   # qT
                    (2, 0, 1), (2, 1, 1)]                          # vT
            # copy order: unblock scores (kT j0, qT j2) and v transposes first
            jcopy = [0, 2, 6, 7, 3, 4, 5, 1]
            jpair = list(range(_NJ))

            def proj_copy(j):
                wi, pt, half = jmap[j]
                src = psums[pt][half * _H:(half + 1) * _H, :]
                b = bs_sb[:, wi:wi + 1]
                if j < 2:
                    dsts = [kqT_sb[r * _H:(r + 1) * _H, j * 512:(j + 1) * 512]
                            for r in range(nrow)]
                elif j < 6:
                    dsts = [kqT_sb[r * _H:(r + 1) * _H,
                                   _SK + (j - 2) * 512:_SK + (j - 1) * 512]
                            for r in range(nrow)]
                else:
                    dsts = [vT_sb[:, (j - 6) * 512:(j - 5) * 512]]
                # split the copy-back between DVE and ACT so the
                # projection->scores handoff isn't serialized on one engine
                for r, dst in enumerate(dsts):
                    if (j + r) % 2 == 0:
                        nc.vector.tensor_scalar_add(dst, src, b)
                    else:
                        nc.scalar.activation(dst, src, Ident, bias=b)

            with (
                tc.tile_pool(name="mm_ps", bufs=4, space="PSUM") as mm_ps,
                tc.tile_pool(name="vt_ps", bufs=2, space="PSUM") as vt_ps,
                tc.tile_pool(name="av_ps", bufs=2, space="PSUM") as av_ps,
                tc.tile_pool(name="expT_p", bufs=1) as expT_p,
            ):
                psums = [mm_ps.tile([_P, 512], f32, name=f"pkq{t}", tag="ps")
                         for t in range(4)]
                xa_tiles, xb_tiles = [], []
                for e in range(_EC):
                    ta = xa_p.tile([_P, _HW], dt_kq, name="xa_t", tag="xa_t")
                    if e == 0:
                        nc.sync.dma_start(ta[:, :512], x_d.ap()[e, :, :512])
                        nc.sync.dma_start(ta[:, 512:], x_d.ap()[e, :, 512:_HW])
                    else:
                        nc.sync.dma_start(ta[:], x_d.ap()[e, :, : _HW])
                    xa_tiles.append(ta)
                    tb = xb_p.tile([_P, _HW], dt_kq, name="xb_t", tag="xb_t")
                    nc.sync.dma_start(tb[:], x_d.ap()[e, :, _HW:])
                    xb_tiles.append(tb)

                def rhs(e, j):
                    if j < 4:
                        return xa_tiles[e][:, j * 512:(j + 1) * 512]
                    return xb_tiles[e][:, (j - 4) * 512:(j - 3) * 512]

                for e in range(_EC):
                    order = jcopy if e == _EC - 1 else jpair
                    for j in order:
                        wi, pt, half = jmap[j]
                        nc.tensor.matmul(
                            psums[pt][half * _H:(half + 1) * _H, :],
                            w_sb[:, wi, e, :],
                            rhs(e, j),
                            start=(e == 0), stop=(e == _EC - 1),
                            tile_position=(0, half * _H) if _PJ_PACK else None,
                            # top/bottom halves of one bank hold independent
                            # accumulation groups (disjoint partitions); the
                            # sim's zero-region tracker can't see that
                            skip_group_check=True,
                        )
                        if e == _EC - 1:
                            proj_copy(j)

                # v[sk, h] via PE transpose of vT
                for skc in range(_SKC):
                    pvt = vt_ps.tile([_P, _H], f32, name="pvt")
                    nc.tensor.transpose(
                        pvt[:],
                        vT_sb[:, skc * _P:(skc + 1) * _P],
                        id_sb[:],
                    )
                    nc.vector.tensor_copy(v_sb[:, skc, : _H], pvt[:])

                # ---- phase 2: scores + exp + AV ----
                expT = expT_p.tile([_P, _SKC, _S], dt_av)
                for skc in range(0, _SKC, nrow):
                    for h in range(_NH):
                        for r in range(nrow):
                            sk = skc + r
                            ps = mm_ps.tile([_P, 512], f32, name="ps", tag="ps")
                            nc.tensor.matmul(
                                ps[:],
                                kqT_sb[r * _H:(r + 1) * _H,
                                       sk * _P:(sk + 1) * _P],
                                kqT_sb[r * _H:(r + 1) * _H,
                                       _SK + h * 512:_SK + (h + 1) * 512],
                                start=True, stop=True,
                                tile_position=(r * _H, 0) if _SC_PACK else None,
                            )
                            nc.scalar.activation(
                                expT[:, sk, h * 512:(h + 1) * 512], ps[:], Exp
                            )

                for h in range(_NH):
                    po = av_ps.tile([_H + 1, 512], f32, name="po")
                    for skc in range(_SKC):
                        nc.tensor.matmul(
                            po[:],
                            v_sb[:, skc, :],
                            expT[:, skc, h * 512:(h + 1) * 512],
                            start=(skc == 0), stop=(skc == _SKC - 1),
                        )
                    nc.vector.tensor_copy(oT_sb[:, h * 512:(h + 1) * 512], po[:])
                    nc.sync.dma_start(
                        out_d.ap()[:, h * 512:(h + 1) * 512],
                        oT_sb[:, h * 512:(h + 1) * 512],
                    )

    nc.compile()
    return nc


def _prep_core(query, key, value, Wq, bq, Wk, bk, Wv, bv, core):
    b, half = core // 2, core % 2
    xkT = np.ascontiguousarray(key[b].T[:, half * _SK:(half + 1) * _SK])
    xqT = np.ascontiguousarray(query[b].T)                 # [E, S]
    xvT = np.ascontiguousarray(value[b].T[:, half * _SK:(half + 1) * _SK])
    x = np.concatenate(
        [xkT.reshape(_EC, _P, _SK), xqT.reshape(_EC, _P, _S),
         xvT.reshape(_EC, _P, _SK)], axis=2,
    )                                                      # [EC, P, W]
    w = np.stack(
        [Wk.reshape(_EC, _P, _H), Wq.reshape(_EC, _P, _H),
         Wv.reshape(_EC, _P, _H)], axis=0,
    ).transpose(2, 0, 1, 3)                                # [P, 3, EC, H]
    bs = np.stack(
        [np.asarray(bk, dtype=np.float32).ravel(),
         np.asarray(bq, dtype=np.float32).ravel(),
         np.asarray(bv, dtype=np.float32).ravel()], axis=1,
    )                                                      # [H, 3]
    return {
        "x": np.ascontiguousarray(x, dtype=np.float32),
        "w": np.ascontiguousarray(w, dtype=np.float32),
        "bs": np.ascontiguousarray(bs, dtype=np.float32),
        "ident": np.eye(_H, dtype=np.float32),
    }


def _get_built():
    global _built
    if _built is None:
        _built = _build()
    return _built


def kernel(query, key, value, Wq, bq, Wk, bk, Wv, bv, _trace=False):
    from concourse.bass_utils import run_bass_kernel_spmd

    query = np.asarray(query, dtype=np.float32)
    key = np.asarray(key, dtype=np.float32)
    value = np.asarray(value, dtype=np.float32)
    Wq = np.asarray(Wq, dtype=np.float32)
    Wk = np.asarray(Wk, dtype=np.float32)
    Wv = np.asarray(Wv, dtype=np.float32)

    nc = _get_built()
    in_maps = [
        _prep_core(query, key, value, Wq, bq, Wk, bk, Wv, bv, c) for c in range(8)
    ]
    res = run_bass_kernel_spmd(nc, in_maps, core_ids=list(range(8)), trace=_trace)
    out = np.empty((_B, _S, _H), dtype=np.float32)
    for b in range(_B):
        oA = res.results[2 * b]["out"]      # [H+1, S]
        oB = res.results[2 * b + 1]["out"]
        num = oA[: _H] + oB[: _H]
        den = oA[_H] + oB[_H]
        out[b] = (num / den).T
    if _trace:
        kernel.last_result = res
    return out
